# revision 14
# baseline (speedup 1.0000x reference)
"""Trainium2 Bass kernel for nn_BidirectionalLayerNeural (gnn_message_passing).

Bidirectional point-cloud cross layer:
  per direction: neural distance matrix [N1,N2] (cosine-of-projected-feats +
  squared euclid), top-k=16 smallest per row, gather neighbor feats/xyz,
  1x1 convs + leaky relu, max-pool over k.

Sharding: rows (query points) split across 8 cores; tables replicated.
Each core runs an identical program on its row shard for both directions.

Self-contained: hardcodes all shapes; host side only slices/repacks inputs.
"""
import numpy as np
from contextlib import ExitStack

import concourse.bass as bass
import concourse.tile as tile
from concourse import bacc, mybir
from concourse import bass_utils

F32 = mybir.dt.float32
I32 = mybir.dt.int32
I16 = mybir.dt.int16
U32 = mybir.dt.uint32
AF = mybir.ActivationFunctionType
OP = mybir.AluOpType
AX = mybir.AxisListType

N = 8192          # total points per cloud
NCORES = 8
NSH = N // NCORES # 1024 query rows per core per direction
C0 = 64           # feature channels
E = 128           # neural-dist embedding dim
KNN = 16
RT = 128          # query rows per tile
TILES = NSH // RT # 8
CH = 512          # distance-matrix column chunk (one PSUM bank)
NCH = N // CH     # 16
SUB = 512         # top-8 subchunk for max8 (assumes <=8 of global top-16 per subchunk)
SLOTS = (N // SUB) * 8  # 256 candidate slots


def _build_phase(tc, H, CONST, POOLS, s):
    """One direction: queries (qx,qf) vs replicated table (tx,tf)."""
    nc = tc.nc
    qx, qf = H[f"qx_{s}"].ap(), H[f"qf_{s}"].ap()
    tx, tf = H[f"tx_{s}"].ap(), H[f"tf_{s}"].ap()
    udram = H[f"udram_{s}"]
    o = H[f"o_{s}"].ap()

    t11T, distT = CONST["t11T"], CONST["distT"]
    poswraw = CONST["poswraw"]
    uprojlhs, q1tlhs = CONST["uprojlhs"], CONST["q1tlhs"]
    w0a, w0b, mlp0bcol = CONST["w0a"], CONST["w0b"], CONST["mlp0bcol"]
    negcol = CONST["negcol"]
    i64, i128, chunkp1 = CONST["i64"], CONST["i128"], CONST["chunkp1"]
    i64r = CONST["i64r"]
    ones128c, ones3c = CONST["ones128c"], CONST["ones3c"]
    ones8k, neg8k, ones1k = CONST["ones8k"], CONST["neg8k"], CONST["ones1k"]

    pb, pp, st, sm, dps, utp, mps, msc = POOLS

    # ---------------- residents (phase-long) ----------------
    BF16 = mybir.dt.bfloat16
    g2 = pb.tile([E, N], F32, tag="g2")            # normalized table embedding
    g1n = pb.tile([E, NSH], F32, tag="g1n")        # normalized query embedding
    q1t = pb.tile([C0, NSH], mybir.dt.float32r, tag="q1t")  # q1+pos_b-WP@x1
    distT68 = pb.tile([68, E], F32, tag="distT68") # [dist_wT; dist_b; -(DW@WP)^T]
    # euclid column terms as one exact-bf16 K=30 matmul:
    # rows 0-26: host 3-way bf16 splits of (2*x1_d) x (x2_d); rows 27-29:
    # ones (lhs) x device bf16 splits of -|x2|^2 (rhs). Per-row terms of the
    # distance are dropped (constant per row -> same top-k).
    k30lhs = pb.tile([30, NSH], BF16, tag="k30lhs")
    k30rhs = pb.tile([30, N], BF16, tag="k30rhs")

    if True:
        work68 = pp.tile([68, N], F32, tag="work68")  # [tf->U; ones; tx]
        f1a68 = pp.tile([68, NSH], F32, tag="f1a68")  # [f1a; ones; 2*x1]
        dptn = pp.tile([3, E], F32, tag="dptn")

        # ---------------- loads ----------------
        nc.sync.dma_start(work68[0:64, :], tf)
        nc.sync.dma_start(work68[64:65, :], ones8k)
        nc.sync.dma_start(work68[65:68, :], tx)

        # distT68 = [distT; -(dist_w @ pos_w)^T]
        nc.sync.dma_start(distT68[0:65, :], distT[:])
        pdp = msc.tile([128, CH], F32, tag="msc")
        nc.tensor.matmul(pdp[0:3, 0:E], poswraw[:], distT68[0:64, :],
                         start=True, stop=True)
        nc.scalar.mul(dptn[:], pdp[0:3, 0:E], -1.0)
        nc.sync.dma_start(distT68[65:68, :], dptn[:])

        # ------------- table: U = t22(tf) + WP@tx (overwrites work68 feats) ----
        for c in range(NCH):
            sl = slice(c * CH, (c + 1) * CH)
            pu = msc.tile([128, CH], F32, tag="msc")
            nc.tensor.matmul(pu[0:C0, :], uprojlhs[:], work68[:, sl],
                             start=True, stop=True)
            nc.scalar.copy(work68[0:C0, sl], pu[0:C0, :])

        # ---------------- U -> DRAM transposed [N, C0] ----------------
        for g in range(8):
            tst = st.tile([128, 8, C0], F32, tag="tst")
            for j in range(8):
                b = g * 8 + j
                pt = msc.tile([128, CH], F32, tag="msc")
                nc.tensor.transpose(pt[:, 0:C0],
                                    work68[0:C0, b * 128:(b + 1) * 128], i64[:])
                nc.scalar.copy(tst[:, j, :], pt[:, 0:C0])
            nc.sync.dma_start(
                udram.ap()[g * 1024:(g + 1) * 1024, :]
                .rearrange("(j p) c -> p j c", p=128),
                tst[:],
            )

        # ------------- g2 = normalize(dist(U) - DP@tx), chunk-pipelined -------
        # raw g2 chunk stays in PSUM; sq/colsum/sqrt/recip/broadcast happen per
        # chunk, then one DVE TT multiplies psum*inv into the g2 resident.
        for c in range(NCH):
            sl = slice(c * CH, (c + 1) * CH)
            pg = dps.tile([RT, CH], F32, tag="dch")
            nc.tensor.matmul(pg[:], distT68[:], work68[:, sl], start=True, stop=True)
            sq = st.tile([E, CH], F32, tag="sqst")
            nc.scalar.square(sq[:], pg[:])
            pn = msc.tile([128, CH], F32, tag="msc")
            nc.tensor.matmul(pn[0:1, :], ones128c[:], sq[:], start=True, stop=True)
            nr = st.tile([1, CH], F32, tag="nrch")
            nc.scalar.sqrt(nr[:], pn[0:1, :])
            nc.vector.tensor_scalar_add(nr[:], nr[:], 1e-8)
            nc.vector.reciprocal(nr[:], nr[:])
            br = st.tile([E, CH], F32, tag="brst")
            nc.gpsimd.partition_broadcast(br[:], nr[:])
            nc.vector.tensor_tensor(g2[:, sl], pg[:], br[:], op=OP.mult)

        # ---------------- query side ----------------
        qf65 = pp.tile([65, NSH], F32, tag="qf65")
        nc.sync.dma_start(qf65[0:64, :], qf)
        nc.sync.dma_start(qf65[64:65, :], ones1k)
        for c in range(2):
            sl = slice(c * CH, (c + 1) * CH)
            pq = msc.tile([128, CH], F32, tag="msc")
            nc.tensor.matmul(pq[0:C0, :], t11T[:], qf65[:, sl], start=True, stop=True)
            nc.scalar.copy(f1a68[0:C0, sl], pq[0:C0, :])
        nc.sync.dma_start(f1a68[64:65, :], ones1k)
        nc.sync.dma_start(f1a68[65:68, :], qx)

        for c in range(2):
            sl = slice(c * CH, (c + 1) * CH)
            pq = dps.tile([RT, CH], F32, tag="dch")
            nc.tensor.matmul(pq[:, :], distT68[0:65, :], f1a68[0:65, sl],
                             start=True, stop=True)
            sq = st.tile([E, CH], F32, tag="sqst")
            nc.scalar.square(sq[:], pq[:])
            pn = msc.tile([128, CH], F32, tag="msc")
            nc.tensor.matmul(pn[0:1, :], ones128c[:], sq[:], start=True, stop=True)
            nr = st.tile([1, CH], F32, tag="nrch")
            nc.scalar.sqrt(nr[:], pn[0:1, :])
            nc.vector.tensor_scalar_add(nr[:], nr[:], 1e-8)
            nc.vector.reciprocal(nr[:], nr[:])
            br = st.tile([E, CH], F32, tag="brst")
            nc.gpsimd.partition_broadcast(br[:], nr[:])
            nc.vector.tensor_tensor(g1n[:, sl], pq[:], br[:], op=OP.mult)

        # -|x2|^2 from host-transposed coords, split into 3 exact bf16 pieces
        txt = H[f"txt_{s}"].ap()
        xt2 = st.tile([128, 64, 3], F32, tag="xt2")
        nc.sync.dma_start(xt2[:], txt)
        xt2s = st.tile([128, 64, 3], F32, tag="xt2s")
        nc.scalar.square(xt2s[:], xt2[:])
        r3w = st.tile([128, 64], F32, tag="r3w")
        nc.vector.tensor_reduce(r3w[:], xt2s[:], axis=AX.X, op=OP.add)
        r3wn = st.tile([128, 64], F32, tag="r3wn")
        nc.scalar.mul(r3wn[:], r3w[:], -1.0)
        res = r3wn
        for piece in range(3):
            pbf = st.tile([128, 64], BF16, tag=f"pbf{piece}")
            nc.vector.tensor_copy(pbf[:], res[:])
            nc.sync.dma_start(k30rhs[27 + piece:28 + piece, :], pbf[:])
            if piece < 2:
                pf = st.tile([128, 64], F32, tag=f"pf{piece}")
                nc.vector.tensor_copy(pf[:], pbf[:])
                nres = st.tile([128, 64], F32, tag=f"nres{piece}")
                nc.vector.tensor_tensor(nres[:], res[:], pf[:], op=OP.subtract)
                res = nres
        # host-packed coordinate split rows (emitted after other loads so a
        # stalled slot-reuse wait cannot block them)
        nc.sync.dma_start(k30lhs[:], H[f"k30lhs_{s}"].ap())
        nc.sync.dma_start(k30rhs[0:27, :], H[f"k30rhs_{s}"].ap())

        # q1t = f1a + pos_b - WP@x1  (lhsT = [I; pos_b; -0.5*WP^T], rhs has 2*x1)
        for c in range(2):
            sl = slice(c * CH, (c + 1) * CH)
            pq = msc.tile([128, CH], F32, tag="msc")
            nc.tensor.matmul(pq[0:C0, :], q1tlhs[:], f1a68[:, sl],
                             start=True, stop=True)
            nc.scalar.copy(q1t[:, sl], pq[0:C0, :])
    # ---------------- tile loop (software-pipelined: dist/topk stage of tile
    # t+1 is emitted before the gather-dependent tail of tile t, so the PE
    # stream never stalls on the topk->gather round trip) ----------------
    def dist_topk_stage(t):
        rsl = slice(t * RT, (t + 1) * RT)
        m8 = sm.tile([RT, SLOTS], F32, tag="m8")
        ix8 = sm.tile([RT, SLOTS], U32, tag="ix8")
        for c in range(NCH):
            csl = slice(c * CH, (c + 1) * CH)
            d = dps.tile([RT, CH], F32, tag="dch")
            # D' = cos - sq - 1 (negated distance; we take top-16 largest)
            nc.tensor.matmul(d[:], g1n[:, rsl], g2[:, csl], start=True, stop=False)
            nc.tensor.matmul(d[:], k30lhs[:, rsl], k30rhs[:, csl],
                             start=False, stop=True)
            for h in range(CH // SUB):
                s8 = c * (CH // SUB) + h
                nc.vector.max(m8[:, s8 * 8:(s8 + 1) * 8],
                              d[:, h * SUB:(h + 1) * SUB])
                nc.vector.max_index(ix8[:, s8 * 8:(s8 + 1) * 8],
                                    m8[:, s8 * 8:(s8 + 1) * 8],
                                    d[:, h * SUB:(h + 1) * SUB])
        # merge: v16 = 16th largest value
        w1 = sm.tile([RT, 8], F32, tag="w1")
        m8r = sm.tile([RT, SLOTS], F32, tag="m8r")
        w2 = sm.tile([RT, 8], F32, tag="w2")
        nc.vector.max(w1[:], m8[:])
        nc.vector.match_replace(m8r[:], w1[:], m8[:], -3.0e38)
        nc.vector.max(w2[:], m8r[:])
        # slot -> global index (+1); mask out non-winners; extract 16 winner idx
        g8 = sm.tile([RT, SLOTS], I32, tag="g8")
        nc.vector.tensor_tensor(g8[:], ix8[:].bitcast(I32), chunkp1[:], op=OP.add)
        g8f = sm.tile([RT, SLOTS], F32, tag="g8f")
        nc.vector.tensor_copy(g8f[:], g8[:])
        nc.vector.tensor_scalar(m8[:], m8[:], w2[:, 7:8], None, op0=OP.is_ge)
        nc.vector.tensor_tensor(g8f[:], m8[:], g8f[:], op=OP.mult)
        nc.vector.tensor_scalar_add(g8f[:], g8f[:], -1.0)
        gix = sm.tile([RT, 16], F32, tag="gix")
        ar = sm.tile([RT, SLOTS], F32, tag="ar")
        nc.vector.max(gix[:, 0:8], g8f[:])
        nc.vector.match_replace(ar[:], gix[:, 0:8], g8f[:], -2.0)
        nc.vector.max(gix[:, 8:16], ar[:])
        # replicate across 8 groups of 16 and transpose via PE -> wrapped layout
        gix8 = sm.tile([RT, 128], F32, tag="gix8")
        nc.vector.tensor_copy(gix8[:], gix[:].unsqueeze(1).broadcast_to([RT, 8, 16]))
        pidx = msc.tile([128, CH], F32, tag="msc")
        nc.tensor.matmul(pidx[:, 0:128], gix8[:], i128[:], start=True, stop=True)
        idx16 = sm.tile([128, 128], I16, tag="idx16")
        nc.vector.tensor_copy(idx16[:], pidx[:, 0:128])

        # gather U rows (token-major halves)
        gA = sm.tile([128, 8, C0], F32, tag="gA")
        gB = sm.tile([128, 8, C0], F32, tag="gB")
        qa, qb = (0, 1) if t % 2 == 0 else (2, 3)
        nc.gpsimd.dma_gather(gA[:], udram.ap(), idx16[:, 0:64],
                             num_idxs=1024, num_idxs_reg=1024, elem_size=C0,
                             queue_num=qa)
        nc.gpsimd.dma_gather(gB[:], udram.ap(), idx16[:, 64:128],
                             num_idxs=1024, num_idxs_reg=1024, elem_size=C0,
                             queue_num=qb)
        return gA, gB

    def tail_stage(t, gAB):
        rsl = slice(t * RT, (t + 1) * RT)
        gA, gB = gAB
        mp = sm.tile([C0, RT], F32, tag="mpool")
        for half, gT in ((0, gA), (1, gB)):
            for bb in range(2):
                b = half * 2 + bb
                pu = utp.tile([C0, CH], F32, tag="ut")
                for jj in range(4):
                    j = bb * 4 + jj
                    nc.tensor.transpose(pu[:, jj * 128:(jj + 1) * 128],
                                        gT[:, j, :], i128[:])
                # s = U[idx] + q1t  (feature-major now)
                r0 = t * RT + b * 32
                ssb = sm.tile([C0, CH], mybir.dt.float32r, tag="ssb")
                nc.vector.tensor_tensor(
                    ssb[:].rearrange("p (r k) -> p r k", k=KNN),
                    pu[:].rearrange("p (r k) -> p r k", k=KNN),
                    q1t[:, r0:r0 + 32].unsqueeze(2).broadcast_to([C0, 32, KNN]),
                    op=OP.add)
                rsb = sm.tile([C0, CH], mybir.dt.float32r, tag="rsb")
                nc.scalar.activation(rsb[:], ssb[:], AF.Relu)
                # mlp0 @ leaky(s) = (0.1*W0)@s + (0.9*W0)@relu(s)
                pm = mps.tile([C0, CH], F32, tag="mp")
                nc.tensor.matmul(pm[:], w0a[:], ssb[:], start=True, stop=False)
                nc.tensor.matmul(pm[:], w0b[:], rsb[:], start=False, stop=True)
                nc.vector.tensor_reduce(
                    mp[:, b * 32:(b + 1) * 32],
                    pm[:].rearrange("p (r k) -> p r k", k=KNN),
                    axis=AX.X, op=OP.max)
        # out = leaky(maxpool + bias)
        yt = sm.tile([C0, RT], F32, tag="yt")
        nc.scalar.activation(yt[:], mp[:], AF.Identity, bias=mlp0bcol[:])
        y01 = sm.tile([C0, RT], F32, tag="y01")
        nc.vector.tensor_scalar_mul(y01[:], yt[:], 0.1)
        ot = sm.tile([C0, RT], F32, tag="ot")
        nc.vector.tensor_tensor(ot[:], yt[:], y01[:], op=OP.max)
        nc.sync.dma_start(o[:, rsl], ot[:])

    pend = [dist_topk_stage(0), dist_topk_stage(1)]
    for t in range(2, TILES):
        nxt = dist_topk_stage(t)
        tail_stage(t - 2, pend[0])
        pend = [pend[1], nxt]
    tail_stage(TILES - 2, pend[0])
    tail_stage(TILES - 1, pend[1])


def build():
    nc = bacc.Bacc("TRN2", target_bir_lowering=False, debug=False,
                   num_devices=NCORES, num_swdge_queues=4)
    H = {}
    for s in ("a", "b"):
        H[f"qx_{s}"] = nc.dram_tensor(f"qx_{s}", [3, NSH], F32, kind="ExternalInput")
        H[f"qf_{s}"] = nc.dram_tensor(f"qf_{s}", [C0, NSH], F32, kind="ExternalInput")
        H[f"tx_{s}"] = nc.dram_tensor(f"tx_{s}", [3, N], F32, kind="ExternalInput")
        H[f"tf_{s}"] = nc.dram_tensor(f"tf_{s}", [C0, N], F32, kind="ExternalInput")
        H[f"udram_{s}"] = nc.dram_tensor(f"udram_{s}", [N, C0], F32, kind="Internal")
        H[f"txt_{s}"] = nc.dram_tensor(f"txt_{s}", [128, 64, 3], F32, kind="ExternalInput")
        H[f"k30lhs_{s}"] = nc.dram_tensor(f"k30lhs_{s}", [30, NSH], mybir.dt.bfloat16,
                                          kind="ExternalInput")
        H[f"k30rhs_{s}"] = nc.dram_tensor(f"k30rhs_{s}", [27, N], mybir.dt.bfloat16,
                                          kind="ExternalInput")
        H[f"o_{s}"] = nc.dram_tensor(f"o_{s}", [C0, NSH], F32, kind="ExternalOutput")
    cshape = {
        "t11T": [65, C0], "distT": [65, E],
        "uprojlhs": [68, C0], "q1tlhs": [68, C0], "poswraw": [C0, 3],
        "mlp0T": [C0, C0], "mlp0bcol": [C0, 1],
        "i64": [C0, C0], "i128": [128, 128],
        "ones128c": [128, 1], "ones3c": [3, 1], "negcol": [128, 1],
        "ones8k": [1, N], "neg8k": [1, N], "ones1k": [1, NSH],
    }
    for k, shp in cshape.items():
        H[k] = nc.dram_tensor(k, shp, F32, kind="ExternalInput")
    H["chunkp1"] = nc.dram_tensor("chunkp1", [128, SLOTS], I32, kind="ExternalInput")

    with tile.TileContext(nc) as tc:
        with ExitStack() as cctx:
            cpool = cctx.enter_context(tc.tile_pool(name="consts", bufs=1))
            CONST = {}
            for k, shp in cshape.items():
                if k in ("ones8k", "neg8k", "ones1k"):
                    CONST[k] = H[k].ap()  # DMA'd straight from DRAM where needed
                    continue
                tl = cpool.tile(shp, F32, tag=k)
                nc.sync.dma_start(tl[:], H[k].ap())
                CONST[k] = tl
            tl = cpool.tile([128, SLOTS], I32, tag="chunkp1")
            nc.sync.dma_start(tl[:], H["chunkp1"].ap())
            CONST["chunkp1"] = tl
            # derived consts (fp32r for the value-path matmuls)
            i64r = cpool.tile([C0, C0], mybir.dt.float32r, tag="i64r")
            nc.scalar.mul(i64r[:], CONST["i64"][:], 1.0)
            CONST["i64r"] = i64r
            w0a = cpool.tile([C0, C0], mybir.dt.float32r, tag="w0a")
            w0b = cpool.tile([C0, C0], mybir.dt.float32r, tag="w0b")
            nc.scalar.mul(w0a[:], CONST["mlp0T"][:], 0.1)
            nc.scalar.mul(w0b[:], CONST["mlp0T"][:], 0.9)
            CONST["w0a"], CONST["w0b"] = w0a, w0b

            with ExitStack() as pools_ctx:
                e = pools_ctx.enter_context
                POOLS = (
                    e(tc.tile_pool(name="pb", bufs=1)),
                    e(tc.tile_pool(name="pp", bufs=1)),
                    e(tc.tile_pool(name="st", bufs=2)),
                    e(tc.tile_pool(name="sm", bufs=3)),
                    e(tc.tile_pool(name="dps", bufs=3, space="PSUM")),
                    e(tc.tile_pool(name="utp", bufs=2, space="PSUM")),
                    e(tc.tile_pool(name="mps", bufs=1, space="PSUM")),
                    e(tc.tile_pool(name="msc", bufs=2, space="PSUM")),
                )
                for s in ("a", "b"):
                    _build_phase(tc, H, CONST, POOLS, s)

    nc.compile()
    return nc, H


def make_in_maps(pc1, pc2, feat1, feat2, t11_w, t11_b, t22_w, t22_b,
                 pos_w, pos_b, dist_w, dist_b, mlp0_w, mlp0_b):
    f = np.float32
    consts = {
        "t11T": np.concatenate([t11_w.T, t11_b[None, :]], 0).astype(f),
        "distT": np.concatenate([dist_w.T, dist_b[None, :]], 0).astype(f),
        "uprojlhs": np.concatenate([t22_w.T, t22_b[None, :], pos_w.T], 0).astype(f),
        "q1tlhs": np.concatenate([np.eye(C0, dtype=f), pos_b[None, :],
                                  -pos_w.T], 0).astype(f),
        "poswraw": np.ascontiguousarray(pos_w).astype(f),
        "mlp0T": np.ascontiguousarray(mlp0_w.T).astype(f),
        "mlp0bcol": mlp0_b[:, None].astype(f),
        "i64": np.eye(C0, dtype=f),
        "i128": np.eye(128, dtype=f),
        "ones128c": np.ones([128, 1], f),
        "ones3c": np.ones([3, 1], f),
        "negcol": -np.ones([128, 1], f),
        "ones8k": np.ones([1, N], f),
        "neg8k": -np.ones([1, N], f),
        "ones1k": np.ones([1, NSH], f),
        "chunkp1": np.tile((np.repeat(np.arange(N // SUB, dtype=np.int32) * SUB, 8)
                            + 1)[None, :], (128, 1)),
    }
    import ml_dtypes
    bf = ml_dtypes.bfloat16

    def split3(v):
        a = v.astype(bf)
        r = (v - a.astype(f)).astype(f)
        b = r.astype(bf)
        c = (r - b.astype(f)).astype(f).astype(bf)
        return a, b, c

    def k30_pair(x1, x2):
        # x1 [3, n1] query coords, x2 [3, N] table coords ->
        # lhs [27, n1] bf16, rhs [27, N] bf16 with sum_k lhs[k]x rhs[k]
        # == sum_d 2*x1_d*x2_d (exactly, via 3x3 split products)
        lhs_p = [split3(2.0 * x1[d]) for d in range(3)]
        rhs_p = [split3(x2[d]) for d in range(3)]
        lhs_rows, rhs_rows = [], []
        for d in range(3):
            for i in range(3):
                for j in range(3):
                    lhs_rows.append(lhs_p[d][i])
                    rhs_rows.append(rhs_p[d][j])
        return np.stack(lhs_rows), np.stack(rhs_rows)

    in_maps = []
    for c in range(NCORES):
        sl = slice(c * NSH, (c + 1) * NSH)
        m = dict(consts)
        m["qx_a"] = np.ascontiguousarray(pc1[0, :, sl])
        m["qf_a"] = np.ascontiguousarray(feat1[0, :, sl])
        m["tx_a"] = np.ascontiguousarray(pc2[0])
        m["tf_a"] = np.ascontiguousarray(feat2[0])
        m["txt_a"] = np.ascontiguousarray(pc2[0].T.reshape(128, 64, 3))
        m["txt_b"] = np.ascontiguousarray(pc1[0].T.reshape(128, 64, 3))
        la, ra = k30_pair(pc1[0, :, sl].astype(f), pc2[0].astype(f))
        lb, rb = k30_pair(pc2[0, :, sl].astype(f), pc1[0].astype(f))
        ones16 = np.ones([3, NSH], ml_dtypes.bfloat16)
        m["k30lhs_a"] = np.ascontiguousarray(np.concatenate([la, ones16], 0))
        m["k30rhs_a"] = np.ascontiguousarray(ra)
        m["k30lhs_b"] = np.ascontiguousarray(np.concatenate([lb, ones16], 0))
        m["k30rhs_b"] = np.ascontiguousarray(rb)
        m["qx_b"] = np.ascontiguousarray(pc2[0, :, sl])
        m["qf_b"] = np.ascontiguousarray(feat2[0, :, sl])
        m["tx_b"] = np.ascontiguousarray(pc1[0])
        m["tf_b"] = np.ascontiguousarray(feat1[0])
        in_maps.append(m)
    return in_maps


_CACHE = {}


def _get_built():
    if "nc" not in _CACHE:
        _CACHE["nc"], _CACHE["H"] = build()
    return _CACHE["nc"], _CACHE["H"]


def run(inputs, trace=False):
    nc, _ = _get_built()
    in_maps = make_in_maps(**{k: np.asarray(v, dtype=np.float32)
                              for k, v in inputs.items()})
    res = bass_utils.run_bass_kernel_spmd(nc, in_maps,
                                          core_ids=list(range(NCORES)),
                                          trace=trace)
    o1 = np.concatenate([res.results[c]["o_a"] for c in range(NCORES)], axis=1)
    o2 = np.concatenate([res.results[c]["o_b"] for c in range(NCORES)], axis=1)
    return (o1[None], o2[None]), res


def kernel(**inputs):
    (o1, o2), _ = run(inputs, trace=False)
    return o1, o2
